# revision 1
# baseline (speedup 1.0000x reference)
"""TRN2 Bass kernel: MultiHeadSelfAttention (B=4, S=2048, D=1024, H=16, DK=64).

Sharding: 8 cores = 4 batches x 2 head-groups (8 heads each).
Per core: QK path in float32r (TF32-ish, 1 cyc/row), V/P path bf16,
softmax via reduce_max + ACT exp(bias=-max), P^T via DMA-transpose (xbar),
PV with [V|1]-stationary -> [O^T; denom], 1/denom broadcast via gpsimd
partition_broadcast, normalization fused into the O^T eviction multiply,
output projection from O^T, partial Y out.
Host: pre-mask x (zeroed masked rows -> masked keys get score 0 -> exp
underflows to exact 0 like the reference's -1e6), pre-transpose x,
permute W columns to [head][dk], fold 1/sqrt(DK) into WQ; final
abs((Y0+Y1)*mask) on host after summing the two head-group partials.
"""

import os
import numpy as np

B, S, D, H, DK = 4, 2048, 1024, 16, 64
HG = 2            # head groups (tensor-parallel)
HL = H // HG      # heads per core = 8
DH = HL * DK      # 512 per-core head width
KT = D // 128     # 8 contraction tiles
NQ = S // 128     # 16 q tiles
NKC = S // 128    # 16 key chunks
QB = 4            # q blocks
QBW = S // QB     # 512 q block width

_cache = {}


def _build():
    from concourse import bacc
    import concourse.mybir as mybir
    import concourse.tile as tile
    from concourse.masks import make_identity

    f32 = mybir.dt.float32
    f32r = mybir.dt.float32r
    bf16 = mybir.dt.bfloat16
    Exp = mybir.ActivationFunctionType.Exp
    AX = mybir.AxisListType.X

    nc = bacc.Bacc("TRN2", target_bir_lowering=False, debug=False, num_devices=8)

    xT_d = nc.dram_tensor("xT", [D, S], f32, kind="ExternalInput")
    wq_d = nc.dram_tensor("wq", [D, DH], f32, kind="ExternalInput")
    wk_d = nc.dram_tensor("wk", [D, DH], f32, kind="ExternalInput")
    wv_d = nc.dram_tensor("wv", [D, DH], f32, kind="ExternalInput")
    wo_d = nc.dram_tensor("wo", [DH, D], f32, kind="ExternalInput")
    y_d = nc.dram_tensor("y", [S, D], f32, kind="ExternalOutput")

    with tile.TileContext(nc) as tc:
        with (
            tc.tile_pool(name="persist", bufs=1) as pp,
            tc.tile_pool(name="psA", bufs=int(os.environ.get("PSA", "7")), space="PSUM") as psA,
            tc.tile_pool(name="psC", bufs=1, space="PSUM") as psC,
        ):
            qT = pp.tile([128, KT // 2, S], f32r, tag="qT")   # (512,2048) 4 ptiles
            kT = pp.tile([128, KT // 2, S], f32r, tag="kT")
            # V with a ones column per head: blocks of 66 = [V_h(64) | 1 | pad]
            v_sb = pp.tile([128, NKC, HL, 66], bf16, tag="v")
            nc.gpsimd.memset(v_sb[:, :, :, 64:65], 1.0)
            wor = pp.tile([128, 4, D], f32r, tag="wor")
            nc.gpsimd.dma_start(wor[:], wo_d.rearrange("(t p) n -> p t n", p=128))

            # ---- phase 1: projections ----
            with (
                tc.tile_pool(name="ph1x", bufs=1) as px,
                tc.tile_pool(name="ph1w", bufs=10) as pw,
                tc.tile_pool(name="ph1wv", bufs=1) as pwv,
            ):
                xr = px.tile([128, KT, S], f32r, tag="xr")
                nc.gpsimd.dma_start(
                    xr[:], xT_d.rearrange("(t p) s -> p t s", p=128)
                )
                wvr = pwv.tile([128, KT, DH], f32r, tag="wvr")
                nc.gpsimd.dma_start(
                    wvr[:], wv_d.rearrange("(t p) n -> p t n", p=128)
                )
                for w_d, dst in ((wq_d, qT), (wk_d, kT)):
                    for p in range(4):
                        wchs = []
                        for k in range(KT):
                            wch = pw.tile([128, 128], f32r, tag="wch")
                            nc.gpsimd.dma_start(
                                wch[:],
                                w_d[k * 128:(k + 1) * 128, p * 128:(p + 1) * 128],
                            )
                            wchs.append(wch)
                        for n in range(4):
                            ps = psA.tile([128, 512], f32, tag="mm")
                            for k in range(KT):
                                nc.tensor.matmul(
                                    ps[:],
                                    wchs[k][:],
                                    xr[:, k, n * 512:(n + 1) * 512],
                                    start=(k == 0),
                                    stop=(k == KT - 1),
                                )
                            nc.vector.tensor_copy(
                                dst[:, p, n * 512:(n + 1) * 512], ps[:]
                            )
                for sc in range(NKC):
                    psv = psA.tile([128, 512], f32, tag="mm")
                    for k in range(KT):
                        nc.tensor.matmul(
                            psv[:],
                            xr[:, k, sc * 128:(sc + 1) * 128],
                            wvr[:, k, :],
                            start=(k == 0),
                            stop=(k == KT - 1),
                        )
                    nc.vector.tensor_copy(
                        v_sb[:, sc, :, 0:64],
                        psv[:].rearrange("p (h w) -> p h w", w=64),
                    )

            # ---- phase 2: attention + output projection ----
            with (
                tc.tile_pool(name="ptb", bufs=int(os.environ.get("PTB", "2")), space="SBUF") as ptbp,
                tc.tile_pool(name="pexp", bufs=int(os.environ.get("PEXP", "3"))) as pexp,
                tc.tile_pool(name="stats", bufs=4) as st,
                tc.tile_pool(name="oTp", bufs=2) as oTp,
                tc.tile_pool(name="yp", bufs=3) as yp,
            ):
                for qb in range(QB):
                    oT = oTp.tile([128, 4, QBW], f32r, tag="oT")
                    for hh in range(HL):
                        p, r0 = hh // 2, (hh % 2) * 64
                        ptb = ptbp.tile([128, QBW // 128, NKC, 128], bf16, tag="ptb")
                        for il in range(QBW // 128):
                            i = qb * 4 + il
                            sq = []
                            for n in range(4):
                                t = psA.tile([128, 512], f32, tag="mm")
                                nc.tensor.matmul(
                                    t[:],
                                    qT[r0:r0 + DK, p, i * 128:(i + 1) * 128],
                                    kT[r0:r0 + DK, p, n * 512:(n + 1) * 512],
                                    start=True,
                                    stop=True,
                                )
                                sq.append(t)
                            mx4 = st.tile([128, 4], f32, tag="mx4")
                            for n in range(4):
                                nc.vector.reduce_max(
                                    mx4[:, n:n + 1], sq[n][:], axis=AX
                                )
                            nm = st.tile([128, 1], f32, tag="nm")
                            nc.vector.tensor_reduce(
                                nm[:], mx4[:], axis=AX,
                                op=mybir.AluOpType.max, negate=True,
                            )
                            p_sb = pexp.tile([128, S], bf16, tag="p")
                            for n in range(4):
                                nc.scalar.activation(
                                    p_sb[:, n * 512:(n + 1) * 512],
                                    sq[n][:],
                                    Exp,
                                    bias=nm[:],
                                    scale=1.0,
                                )
                            nc.sync.dma_start(
                                ptb[:, il, :, :],
                                p_sb[:],
                                transpose=True,
                            )
                        # PV with [V_h | 1] stationary -> [O^T ; denom-row]
                        ot_ps = psC.tile([65, QBW], f32, tag="ot")
                        for kc in range(NKC):
                            nc.tensor.matmul(
                                ot_ps[:],
                                v_sb[:, kc, hh, 0:65],
                                ptb[:, :, kc, :],
                                start=(kc == 0),
                                stop=(kc == NKC - 1),
                            )
                        # recip of denom row, broadcast to 64 partitions
                        rrow = st.tile([1, QBW], f32, tag="rrow")
                        nc.vector.reciprocal(rrow[:], ot_ps[64:65, :])
                        rb = st.tile([64, QBW], f32, tag="rb")
                        nc.gpsimd.partition_broadcast(rb[:], rrow[:])
                        nc.vector.tensor_mul(
                            oT[r0:r0 + 64, p, :], ot_ps[0:64, :], rb[:]
                        )
                    for il in range(QBW // 128):
                        i = qb * 4 + il
                        y_sb = yp.tile([128, D], f32, tag="y")
                        for half in range(2):
                            yq = psA.tile([128, 512], f32, tag="mm")
                            for pp_ in range(4):
                                nc.tensor.matmul(
                                    yq[:],
                                    oT[:, pp_, il * 128:(il + 1) * 128],
                                    wor[:, pp_, half * 512:(half + 1) * 512],
                                    start=(pp_ == 0),
                                    stop=(pp_ == 3),
                                )
                            nc.scalar.copy(
                                y_sb[:, half * 512:(half + 1) * 512], yq[:])
                        nc.sync.dma_start(y_d[i * 128:(i + 1) * 128, :], y_sb[:])

    nc.compile()
    return nc


def _prep_inputs(x, mask, WQ, WK, WV, WO):
    xm = (x.astype(np.float32) * mask.astype(np.float32)[:, :, None])
    in_maps = []
    for c in range(8):
        b, g = c // 2, c % 2
        idx = np.array(
            [dk * H + (g * HL + hh) for hh in range(HL) for dk in range(DK)]
        )
        in_maps.append({
            "xT": np.ascontiguousarray(xm[b].T),
            "wq": np.ascontiguousarray(WQ[:, idx] / np.sqrt(DK)).astype(np.float32),
            "wk": np.ascontiguousarray(WK[:, idx]).astype(np.float32),
            "wv": np.ascontiguousarray(WV[:, idx]).astype(np.float32),
            "wo": np.ascontiguousarray(WO[g * DH:(g + 1) * DH, :]).astype(np.float32),
        })
    return in_maps


def kernel(x, mask, WQ, WK, WV, WO, _want_results=False, _trace=False):
    from concourse.bass_utils import run_bass_kernel_spmd

    if "nc" not in _cache:
        _cache["nc"] = _build()
    nc = _cache["nc"]
    in_maps = _prep_inputs(np.asarray(x), np.asarray(mask), np.asarray(WQ),
                           np.asarray(WK), np.asarray(WV), np.asarray(WO))
    res = run_bass_kernel_spmd(nc, in_maps, list(range(8)), trace=_trace)
    ys = [res.results[c]["y"] for c in range(8)]
    mk = np.asarray(mask).astype(np.float32)
    out = np.empty((B, S, D), np.float32)
    for b in range(B):
        out[b] = np.abs((ys[2 * b] + ys[2 * b + 1]) * mk[b][:, None])
    if _want_results:
        return out, res
    return out



# revision 8
# speedup vs baseline: 2.6158x; 2.6158x over previous
"""TRN2 Bass kernel: MultiHeadSelfAttention (B=4, S=2048, D=1024, H=16, DK=64).

Key optimization vs the dense version: mask compaction. The reference
multiplies the output by mask (query side) and gives masked keys -1e6
scores (= exactly 0 softmax weight), so attention restricted to the
valid positions only is mathematically identical. Valid counts are
~1024 per batch; we gather valid rows on the host, pad to SP=1152
(9x128), run attention over 1152 positions instead of 2048, and
scatter back. This cuts all S^2 work (scores, exp, max, transposes,
PV) by ~3.2x and the projections by ~1.8x.

Sharding: 8 cores = 4 batches x 2 head-groups (8 heads each).
Per core: QK path f32r, V/P path bf16, softmax via one reduce_max +
one Exp activation (bias=-max) over the whole [128, 1152] score tile,
P^T via DMA-transpose (xbar), PV with [V_h|1]-stationary ->
[O_h^T ; denom], 1/denom via DVE recip + gpsimd partition_broadcast,
normalization fused into the O^T eviction multiply, output projection
from O^T.

f32r matmuls with moving width <256 cost 4 cyc/row, so x/k tiles carry
a 128-col zero guard (SG=1280): the third score/projection chunk runs
256 wide at 1 cyc/row; guard scores are never read by max/exp/PV.

Emission interleaves V-projection chunks and prev-head PV into the
score stream so the PE stream stays dense.
"""

import numpy as np

B, S, D, H, DK = 4, 2048, 1024, 16, 64
HG = 2            # head groups (tensor-parallel)
HL = H // HG      # heads per core = 8
DH = HL * DK      # 512 per-core head width
KT = D // 128     # 8 contraction tiles
SP = 1152         # padded valid positions (9 x 128)
SG = 1280         # guard width for f32r moving chunks (10 x 128)
NQ = SP // 128    # 9 q tiles
NKC = SP // 128   # 9 key chunks
CHUNKS = ((0, 512), (512, 512), (1024, 256))   # within SG, last is guard-wide
PV_CHUNKS = ((0, 512), (512, 512), (1024, 128))  # q chunks within SP

_cache = {}


def _build():
    from concourse import bacc
    import concourse.mybir as mybir
    import concourse.tile as tile

    f32 = mybir.dt.float32
    f32r = mybir.dt.float32r
    bf16 = mybir.dt.bfloat16
    Exp = mybir.ActivationFunctionType.Exp
    AX = mybir.AxisListType.X

    nc = bacc.Bacc("TRN2", target_bir_lowering=False, debug=False, num_devices=8)

    xT_d = nc.dram_tensor("xT", [D, SG], f32, kind="ExternalInput")
    wq_d = nc.dram_tensor("wq", [D, DH], f32, kind="ExternalInput")
    wk_d = nc.dram_tensor("wk", [D, DH], f32, kind="ExternalInput")
    wv_d = nc.dram_tensor("wv", [D, DH], f32, kind="ExternalInput")
    wo_d = nc.dram_tensor("wo", [DH, D], f32, kind="ExternalInput")
    y_d = nc.dram_tensor("y", [SP, D], f32, kind="ExternalOutput")

    with tile.TileContext(nc) as tc:
        with (
            tc.tile_pool(name="persist", bufs=1) as pp,
            tc.tile_pool(name="ps", bufs=1, space="PSUM") as ps,
        ):
            qT = pp.tile([128, 4, SG], f32r, tag="qT")
            kT = pp.tile([128, 4, SG], f32r, tag="kT")
            v_sb = pp.tile([128, NKC, HL, 66], bf16, tag="v")
            wor = pp.tile([128, 4, D], f32r, tag="wor")
            oT = pp.tile([128, 4, SP], f32r, tag="oT")

            nc.gpsimd.memset(v_sb[:, :, :, 64:65], 1.0)
            nc.gpsimd.dma_start(wor[:], wo_d.rearrange("(t p) n -> p t n", p=128))

            # ---------- emission helpers ----------
            def qk_proj_p(w_sb, dst, p, xr):
                # three chunks share one psum tile, then one wide eviction
                pst = ps.tile([128, SG], f32, tag="s", bufs=2)
                for c0, cw in CHUNKS:
                    for k in range(KT):
                        nc.tensor.matmul(
                            pst[:, c0:c0 + cw],
                            w_sb[:, k, p * 128:(p + 1) * 128],
                            xr[:, k, c0:c0 + cw],
                            start=(k == 0),
                            stop=(k == KT - 1),
                        )
                nc.scalar.copy(dst[:, p, :], pst[:])

            def v_proj_chunk(sc, xr, wvr):
                psv = ps.tile([128, 512], f32, tag="o", bufs=1)
                for k in range(KT):
                    nc.tensor.matmul(
                        psv[:],
                        xr[:, k, sc * 128:(sc + 1) * 128],
                        wvr[:, k, :],
                        start=(k == 0),
                        stop=(k == KT - 1),
                    )
                nc.scalar.copy(
                    v_sb[:, sc, :, 0:64],
                    psv[:].rearrange("p (h w) -> p h w", w=64),
                )

            with tc.tile_pool(name="ph1long", bufs=1) as p1:
                xr = p1.tile([128, KT, SG], f32r, tag="xr")
                wvr = p1.tile([128, KT, DH], f32r, tag="wvr")
                nc.gpsimd.dma_start(
                    xr[:], xT_d.rearrange("(t p) s -> p t s", p=128)
                )
                nc.gpsimd.dma_start(
                    wvr[:], wv_d.rearrange("(t p) n -> p t n", p=128)
                )

                with tc.tile_pool(name="ph1w", bufs=1) as pw:
                    wkr = pw.tile([128, KT, DH], f32r, tag="wkr")
                    wqr = pw.tile([128, KT, DH], f32r, tag="wqr")
                    nc.gpsimd.dma_start(
                        wkr[:], wk_d.rearrange("(t p) n -> p t n", p=128)
                    )
                    nc.gpsimd.dma_start(
                        wqr[:], wq_d.rearrange("(t p) n -> p t n", p=128)
                    )
                    # serial prefix: K then Q projections (scores need them)
                    for p in range(4):
                        qk_proj_p(wkr, kT, p, xr)
                    for p in range(4):
                        qk_proj_p(wqr, qT, p, xr)

                with (
                    tc.tile_pool(name="ptbp", bufs=2) as ptbp,
                    tc.tile_pool(name="pexp", bufs=2) as pexp,
                    tc.tile_pool(name="stats", bufs=4) as st,
                    tc.tile_pool(name="yp", bufs=2) as yp,
                ):
                    ptbs = {}

                    def score_qtile(h, i):
                        p, r0 = h // 2, (h % 2) * 64
                        pst = ps.tile([128, SG], f32, tag="s", bufs=2)
                        for c0, cw in CHUNKS:
                            nc.tensor.matmul(
                                pst[:, c0:c0 + cw],
                                qT[r0:r0 + DK, p, i * 128:(i + 1) * 128]
                                ,
                                kT[r0:r0 + DK, p, c0:c0 + cw],
                                start=True,
                                stop=True,
                            )
                        nm = st.tile([128, 1], f32, tag="nm")
                        nc.vector.tensor_reduce(
                            nm[:], pst[:, 0:SP], axis=AX,
                            op=mybir.AluOpType.max, negate=True,
                        )
                        p_sb = pexp.tile([128, SP], bf16, tag="p")
                        nc.scalar.activation(
                            p_sb[:], pst[:, 0:SP], Exp, bias=nm[:], scale=1.0
                        )
                        nc.sync.dma_start(
                            ptbs[h][:, :, i * 128:(i + 1) * 128],
                            p_sb[:],
                            transpose=True,
                        )

                    def pv_chunk(h, qc):
                        p, r0 = h // 2, (h % 2) * 64
                        q0, qw = PV_CHUNKS[qc]
                        ptb = ptbs[h]
                        psv = ps.tile([65, 512], f32, tag="pv", bufs=1)
                        for kc in range(NKC):
                            nc.tensor.matmul(
                                psv[:, 0:qw],
                                v_sb[:, kc, h, 0:65],
                                ptb[:, kc, q0:q0 + qw],
                                start=(kc == 0),
                                stop=(kc == NKC - 1),
                            )
                        rrow = st.tile([1, 512], f32, tag="rrow", bufs=2)
                        nc.vector.reciprocal(rrow[:, 0:qw], psv[64:65, 0:qw])
                        rb = st.tile([64, 512], f32, tag="rb", bufs=2)
                        nc.gpsimd.partition_broadcast(rb[:, 0:qw], rrow[:, 0:qw])
                        nc.vector.tensor_mul(
                            oT[r0:r0 + 64, p, q0:q0 + qw],
                            psv[0:64, 0:qw],
                            rb[:, 0:qw],
                        )

                    def oproj_il(il):
                        y_sb = yp.tile([128, D], f32, tag="y")
                        for half in range(2):
                            yq = ps.tile([128, 512], f32, tag="o", bufs=1)
                            for p in range(4):
                                nc.tensor.matmul(
                                    yq[:],
                                    oT[:, p, il * 128:(il + 1) * 128]
                                    ,
                                    wor[:, p, half * 512:(half + 1) * 512]
                                    ,
                                    start=(p == 0),
                                    stop=(p == 3),
                                )
                            nc.scalar.copy(
                                y_sb[:, half * 512:(half + 1) * 512], yq[:]
                            )
                        nc.sync.dma_start(y_d[il * 128:(il + 1) * 128, :], y_sb[:])

                    # fillers for PE gaps in the softmax-bound score stream
                    fillers = [
                        (lambda sc=sc: v_proj_chunk(sc, xr, wvr))
                        for sc in range(NKC)
                    ]

                    for h in range(HL):
                        ptbs[h] = ptbp.tile(
                            [128, NKC, SP], bf16, tag="ptb", name=f"ptb{h}"
                        )
                        for i in range(NQ):
                            score_qtile(h, i)
                            # drain all V chunks during head 0: pv_chunk(0, *)
                            # must be emitted after every v_sb write (emission
                            # order defines dependency order)
                            if i >= 2:
                                for _ in range(2 if fillers else 0):
                                    if fillers:
                                        fillers.pop(0)()
                            # prev head's PV interleaved into this head's stream
                            if h >= 1 and i in (2, 5, 8):
                                pv_chunk(h - 1, {2: 0, 5: 1, 8: 2}[i])
                    # tail: last head's PV + output projection interleaved
                    pv_chunk(HL - 1, 0)
                    for il in range(4):
                        oproj_il(il)
                    pv_chunk(HL - 1, 1)
                    for il in range(4, 8):
                        oproj_il(il)
                    pv_chunk(HL - 1, 2)
                    oproj_il(8)

    nc.compile()
    return nc


def _prep_inputs(x, mask, WQ, WK, WV, WO):
    idxs = [np.nonzero(mask[b])[0] for b in range(B)]
    assert max(len(ix) for ix in idxs) <= SP, "valid count exceeds padding"
    in_maps = []
    for c in range(8):
        b, g = c // 2, c % 2
        ix = idxs[b]
        xv = np.zeros((SG, D), np.float32)
        xv[: len(ix)] = x[b][ix]
        hidx = np.array(
            [dk * H + (g * HL + hh) for hh in range(HL) for dk in range(DK)]
        )
        in_maps.append({
            "xT": np.ascontiguousarray(xv.T),
            "wq": np.ascontiguousarray(WQ[:, hidx] / np.sqrt(DK)).astype(np.float32),
            "wk": np.ascontiguousarray(WK[:, hidx]).astype(np.float32),
            "wv": np.ascontiguousarray(WV[:, hidx]).astype(np.float32),
            "wo": np.ascontiguousarray(WO[g * DH:(g + 1) * DH, :]).astype(np.float32),
        })
    return in_maps, idxs


def kernel(x, mask, WQ, WK, WV, WO, _want_results=False, _trace=False):
    from concourse.bass_utils import run_bass_kernel_spmd

    if "nc" not in _cache:
        _cache["nc"] = _build()
    nc = _cache["nc"]
    x, mask = np.asarray(x), np.asarray(mask)
    in_maps, idxs = _prep_inputs(
        x.astype(np.float32), mask, np.asarray(WQ), np.asarray(WK),
        np.asarray(WV), np.asarray(WO),
    )
    res = run_bass_kernel_spmd(nc, in_maps, list(range(8)), trace=_trace)
    out = np.zeros((B, S, D), np.float32)
    for b in range(B):
        ix = idxs[b]
        yv = res.results[2 * b]["y"][: len(ix)] + res.results[2 * b + 1]["y"][: len(ix)]
        out[b][ix] = np.abs(yv)
    if _want_results:
        return out, res
    return out


# revision 51
# speedup vs baseline: 2.7821x; 1.0636x over previous
"""TRN2 Bass kernel: MultiHeadSelfAttention (B=4, S=2048, D=1024, H=16, DK=64).

Key optimization vs the dense version: mask compaction. The reference
multiplies the output by mask (query side) and gives masked keys -1e6
scores (= exactly 0 softmax weight), so attention restricted to the
valid positions only is mathematically identical. Valid counts are
~1024 per batch; we gather valid rows on the host, pad to SP=1152
(9x128), run attention over 1152 positions instead of 2048, and
scatter back. This cuts all S^2 work (scores, exp, max, transposes,
PV) by ~3.2x and the projections by ~1.8x.

Sharding: 8 cores = 4 batches x 2 head-groups (8 heads each).
Per core: QK path f32r, V/P path bf16, softmax via one reduce_max +
one Exp activation (bias=-max) over the whole [128, 1152] score tile,
P^T via DMA-transpose (xbar), PV with [V_h|1]-stationary ->
[O_h^T ; denom], 1/denom via DVE recip + gpsimd partition_broadcast,
normalization fused into the O^T eviction multiply, output projection
from O^T. (gpsimd cannot touch PSUM, so PSUM evictions live on
DVE/ACT only.)

f32r matmuls with moving width <256 cost 4 cyc/row, so x/k tiles carry
a 128-col zero guard (SG=1280): the third score/projection chunk runs
256 wide at 1 cyc/row; guard scores are never read by max/exp/PV.

Scheduling (the softmax chain scores->reduce->exp->transpose is
latency-bound; PSUM allows only two 3-bank score slots, so the chain
paces the kernel at ~2.2us per (head, qtile) unit):
 - phase 2 runs per query-block (4/4/1 qtiles): PV consumes each
   block's transposes and the output projection drains one block
   behind, so no work piles into a tail;
 - only K/Q projections for p-block 0 run as a serial prefix; K/Q
   p1-3 (heads 2b need only p-block b) and all V chunks are emitted
   as fillers into the score-gaps, one per score (two during head 0),
   keeping the PE stream dense and the pstate high;
 - filler projections evict through the single-bank "o" psum ring;
   their PSUM->SBUF evictions run on DVE (ACT delays exp, Pool is
   illegal for PSUM);
 - WO reuses WK's SBUF (flat tile, DMA-reloaded after K-proj's last
   read -- emission order encodes the WAR dependency);
 - emission order defines dependency order: all v_sb writes must
   precede the first pv_chunk, K/Q p-block b must precede head 2b
   (enforced via need_before_head).

PSUM: 2x3-bank score slots + 1-bank "o" ring + 1-bank PV = 8 banks.
"""

import numpy as np

B, S, D, H, DK = 4, 2048, 1024, 16, 64
HG = 2            # head groups (tensor-parallel)
HL = H // HG      # heads per core = 8
DH = HL * DK      # 512 per-core head width
KT = D // 128     # 8 contraction tiles
SP = 1152         # padded valid positions (9 x 128)
SG = 1280         # guard width for f32r moving chunks (10 x 128)
NQ = SP // 128    # 9 q tiles
NKC = SP // 128   # 9 key chunks
CHUNKS = ((0, 512), (512, 512), (1024, 256))   # within SG, last is guard-wide
QBS = ((0, 4), (4, 4), (8, 1))   # query blocks: (first qtile, n qtiles)

_cache = {}


def _build():
    from concourse import bacc
    import concourse.mybir as mybir
    import concourse.tile as tile

    f32 = mybir.dt.float32
    f32r = mybir.dt.float32r
    bf16 = mybir.dt.bfloat16
    Exp = mybir.ActivationFunctionType.Exp
    AX = mybir.AxisListType.X

    nc = bacc.Bacc("TRN2", target_bir_lowering=False, debug=False, num_devices=8)

    xT_d = nc.dram_tensor("xT", [D, SG], f32, kind="ExternalInput")
    wq_d = nc.dram_tensor("wq", [D, DH], f32, kind="ExternalInput")
    wk_d = nc.dram_tensor("wk", [D, DH], f32, kind="ExternalInput")
    wv_d = nc.dram_tensor("wv", [D, DH], f32, kind="ExternalInput")
    wo_d = nc.dram_tensor("wo", [DH, D], f32, kind="ExternalInput")
    y_d = nc.dram_tensor("y", [SP, D], f32, kind="ExternalOutput")

    with tile.TileContext(nc) as tc:
        with (
            tc.tile_pool(name="persist", bufs=1) as pp,
            tc.tile_pool(name="ps", bufs=1, space="PSUM") as ps,
            tc.tile_pool(name="ph1", bufs=1) as p1,
            tc.tile_pool(name="ptbp", bufs=3) as ptbp,
            tc.tile_pool(name="pexp", bufs=3) as pexp,
            tc.tile_pool(name="stats", bufs=4) as st,
            tc.tile_pool(name="yp", bufs=2) as yp,
        ):
            qT = pp.tile([128, 4, SP], f32r, tag="qT")
            kT = pp.tile([128, 4, SG], f32r, tag="kT")
            v_sb = pp.tile([128, NKC, HL, 66], bf16, tag="v")
            # WK and WO share this flat tile: K-proj reads the wk view,
            # then the tile is overwritten with WO for the output proj.
            wk_wo = pp.tile([128, 4096], f32r, tag="wk_wo")
            oT = pp.tile([128, 4, SP], f32r, tag="oT")
            wkr = wk_wo.rearrange("p (t n) -> p t n", n=DH)
            wor = wk_wo.rearrange("p (t n) -> p t n", n=D)

            xr = p1.tile([128, KT, SG], f32r, tag="xr")
            wvr = p1.tile([128, KT, DH], f32r, tag="wvr")
            wqr = p1.tile([128, KT, DH], f32r, tag="wqr")

            nc.gpsimd.memset(v_sb[:, :, :, 64:65], 1.0)
            # issue order = arrival order on the exclusive DMA device
            nc.gpsimd.dma_start(
                wkr[:], wk_d.rearrange("(t p) n -> p t n", p=128)
            )
            nc.gpsimd.dma_start(
                xr[:, :, 0:512],
                xT_d[:, 0:512].rearrange("(t p) s -> p t s", p=128),
            )
            nc.gpsimd.dma_start(
                wqr[:], wq_d.rearrange("(t p) n -> p t n", p=128)
            )
            for c0, cw in CHUNKS[1:]:
                nc.gpsimd.dma_start(
                    xr[:, :, c0:c0 + cw],
                    xT_d[:, c0:c0 + cw].rearrange("(t p) s -> p t s", p=128),
                )
            nc.gpsimd.dma_start(
                wvr[:], wv_d.rearrange("(t p) n -> p t n", p=128)
            )

            _EV = {
                "dve": nc.vector.tensor_copy,
                "act": nc.scalar.copy,
                "pool": nc.gpsimd.tensor_copy,
            }

            # ---------- emission helpers ----------
            def filler_tag():
                # before the first pv_chunk the "pv" bank is idle: alternate
                # early fillers across both single-bank rings to double-buffer
                if filler_n[0] < 11:
                    filler_n[0] += 1
                    return "pv" if filler_n[0] % 2 else "o"
                return "o"

            def qk_proj_p(w_sb, dst, p, wide, use_o=False):
                if use_o:
                    # filler path: per-chunk psum in the "o"/"pv" rings so the
                    # score pipeline keeps both of its "s" slots
                    for c0, cw in CHUNKS:
                        pso = ps.tile([128, 512], f32, tag=filler_tag(),
                                      bufs=1, name="pso")
                        for k in range(KT):
                            nc.tensor.matmul(
                                pso[:, 0:cw],
                                w_sb[:, k, p * 128:(p + 1) * 128],
                                xr[:, k, c0:c0 + cw],
                                start=(k == 0),
                                stop=(k == KT - 1),
                            )
                        w = cw if wide else min(cw, SP - c0)
                        _EV["dve"](
                            dst[:, p, c0:c0 + w], pso[:, 0:w]
                        )
                    return
                # prefix path: three chunks share one wide "s" psum tile
                pst = ps.tile([128, SG], f32, tag="s", bufs=2, name="pst")
                for c0, cw in CHUNKS:
                    for k in range(KT):
                        nc.tensor.matmul(
                            pst[:, c0:c0 + cw],
                            w_sb[:, k, p * 128:(p + 1) * 128],
                            xr[:, k, c0:c0 + cw],
                            start=(k == 0),
                            stop=(k == KT - 1),
                        )
                w = SG if wide else SP
                nc.scalar.copy(dst[:, p, 0:w], pst[:, 0:w])

            def v_proj_chunk(sc):
                psv = ps.tile([128, 512], f32, tag=filler_tag(), bufs=1,
                              name="psv")
                for k in range(KT):
                    nc.tensor.matmul(
                        psv[:],
                        xr[:, k, sc * 128:(sc + 1) * 128],
                        wvr[:, k, :],
                        start=(k == 0),
                        stop=(k == KT - 1),
                    )
                _EV["dve"](
                    v_sb[:, sc, :, 0:64],
                    psv[:].rearrange("p (h w) -> p h w", w=64),
                )

            ptbs = {}
            filler_n = [0]
            pend_tr = []

            def flush_transpose():
                while pend_tr:
                    pend_tr.pop(0)()

            def score_qtile(h, i, ptb, ii):
                p, r0 = h // 2, (h % 2) * 64
                pst = ps.tile([128, SG], f32, tag="s", bufs=2, name="pst")
                for c0, cw in CHUNKS:
                    nc.tensor.matmul(
                        pst[:, c0:c0 + cw],
                        qT[r0:r0 + DK, p, i * 128:(i + 1) * 128],
                        kT[r0:r0 + DK, p, c0:c0 + cw],
                        start=True,
                        stop=True,
                    )
                nm = st.tile([128, 1], f32, tag="nm", name="nm")
                nc.vector.tensor_reduce(
                    nm[:], pst[:, 0:SP], axis=AX,
                    op=mybir.AluOpType.max, negate=True,
                )
                p_sb = pexp.tile([128, SP], bf16, tag="p", name="p_sb")
                nc.scalar.activation(
                    p_sb[:], pst[:, 0:SP], Exp, bias=nm[:], scale=1.0
                )
                # defer the transpose dispatch by one qtile: when SP reaches
                # it, the exp has finished, so the SP queue never blocks
                flush_transpose()
                pend_tr.append(lambda: nc.sync.dma_start(
                    ptb[:, :, ii * 128:(ii + 1) * 128],
                    p_sb[:],
                    transpose=True,
                ))

            def pv_chunk(h, qb):
                flush_transpose()
                p, r0 = h // 2, (h % 2) * 64
                i0, ni = QBS[qb]
                q0, qw = i0 * 128, ni * 128
                ptb = ptbs[(h, qb)]
                psv = ps.tile([65, 512], f32, tag="pv", bufs=1, name="pspv")
                for kc in range(NKC):
                    nc.tensor.matmul(
                        psv[:, 0:qw],
                        v_sb[:, kc, h, 0:65],
                        ptb[:, kc, 0:qw],
                        start=(kc == 0),
                        stop=(kc == NKC - 1),
                    )
                rrow = st.tile([1, 512], f32, tag="rrow", bufs=1, name="rrow")
                nc.vector.reciprocal(rrow[:, 0:qw], psv[64:65, 0:qw])
                rb = st.tile([64, 512], f32, tag="rb", bufs=1, name="rb")
                nc.gpsimd.partition_broadcast(rb[:, 0:qw], rrow[:, 0:qw])
                nc.vector.tensor_mul(
                    oT[r0:r0 + 64, p, q0:q0 + qw],
                    psv[0:64, 0:qw],
                    rb[:, 0:qw],
                )

            def oproj_il(il, dma_eng=None):
                dma_eng = dma_eng or nc.sync
                y_sb = yp.tile([128, D], f32, tag="y", name="y_sb")
                for half in range(2):
                    yq = ps.tile([128, 512], f32, tag="o", bufs=1, name="yq")
                    for p in range(4):
                        nc.tensor.matmul(
                            yq[:],
                            oT[:, p, il * 128:(il + 1) * 128],
                            wor[:, p, half * 512:(half + 1) * 512],
                            start=(p == 0),
                            stop=(p == 3),
                        )
                    nc.scalar.copy(y_sb[:, half * 512:(half + 1) * 512], yq[:])
                dma_eng.dma_start(y_d[il * 128:(il + 1) * 128, :], y_sb[:])

            def reload_wo():
                # overwrite the WK tile with WO (all K-proj reads precede
                # this in emission order, so the WAR dependency is tracked)
                nc.gpsimd.dma_start(
                    wor[:], wo_d.rearrange("(t p) n -> p t n", p=128)
                )

            # ---------- emission schedule ----------
            qk_proj_p(wkr, kT, 0, True)
            qk_proj_p(wqr, qT, 0, False)

            # everything else interleaves into the score-pipeline gaps.
            # Constraints encoded by emission order:
            #  - all 9 V chunks before the first pv_chunk (h1 end)
            #  - K/Q p-block b before scores of head 2b (h-loop position)
            #  - WO reload after K p3's last read, before the first oproj
            fillers = [lambda sc=sc: v_proj_chunk(sc) for sc in range(NKC)]
            fillers += [
                lambda: qk_proj_p(wkr, kT, 1, True, use_o=True),
                lambda: qk_proj_p(wqr, qT, 1, False, use_o=True),
                lambda: qk_proj_p(wkr, kT, 2, True, use_o=True),
                lambda: qk_proj_p(wqr, qT, 2, False, use_o=True),
                lambda: qk_proj_p(wkr, kT, 3, True, use_o=True),
                lambda: qk_proj_p(wqr, qT, 3, False, use_o=True),
            ]
            need_before_head = {2: 11, 3: 11, 4: 13, 5: 13, 6: 15, 7: 15}
            fillers.append(reload_wo)
            popped = [0]

            def pop_filler(n=1):
                for _ in range(n):
                    if fillers:
                        fillers.pop(0)()
                        popped[0] += 1

            pending = []   # deferred pv7/oproj units from the previous block

            for qb, (i0, ni) in enumerate(QBS):
                for h in range(HL):
                    need = need_before_head.get(h, 0) - popped[0]
                    if need > 0:
                        pop_filler(need)
                    ptbs[(h, qb)] = ptbp.tile(
                        [128, NKC, 512], bf16, tag="ptb",
                        name=f"ptb{h}_{qb}", bufs=3,
                    )
                    for ii in range(ni):
                        score_qtile(h, i0 + ii, ptbs[(h, qb)], ii)
                        # 2/gap only while no PV competes (head 0); bursts of
                        # fillers on the single "o" psum slot stall the PE
                        pop_filler(2 if (qb == 0 and h == 0) else 1)
                        if pending:
                            pending.pop(0)()
                    if h >= 1:
                        pv_chunk(h - 1, qb)
                # last head's PV of this block runs early in the next block's
                # stream (its transposes lag the last scores); the block's
                # output projection follows once all heads' O^T are in
                pending.append(lambda qb=qb: pv_chunk(HL - 1, qb))
                pending.extend(
                    (lambda il=il: oproj_il(il)) for il in range(i0, i0 + ni)
                )
            while fillers or pending:
                pop_filler()
                if pending:
                    pending.pop(0)()
            flush_transpose()

    nc.compile()
    return nc


def _prep_inputs(x, mask, WQ, WK, WV, WO):
    idxs = [np.nonzero(mask[b])[0] for b in range(B)]
    assert max(len(ix) for ix in idxs) <= SP, "valid count exceeds padding"
    in_maps = []
    for c in range(8):
        b, g = c // 2, c % 2
        ix = idxs[b]
        xv = np.zeros((SG, D), np.float32)
        xv[: len(ix)] = x[b][ix]
        hidx = np.array(
            [dk * H + (g * HL + hh) for hh in range(HL) for dk in range(DK)]
        )
        in_maps.append({
            "xT": np.ascontiguousarray(xv.T),
            "wq": np.ascontiguousarray(WQ[:, hidx] / np.sqrt(DK)).astype(np.float32),
            "wk": np.ascontiguousarray(WK[:, hidx]).astype(np.float32),
            "wv": np.ascontiguousarray(WV[:, hidx]).astype(np.float32),
            "wo": np.ascontiguousarray(WO[g * DH:(g + 1) * DH, :]).astype(np.float32),
        })
    return in_maps, idxs


def kernel(x, mask, WQ, WK, WV, WO, _want_results=False, _trace=False):
    from concourse.bass_utils import run_bass_kernel_spmd

    if "nc" not in _cache:
        _cache["nc"] = _build()
    nc = _cache["nc"]
    x, mask = np.asarray(x), np.asarray(mask)
    in_maps, idxs = _prep_inputs(
        x.astype(np.float32), mask, np.asarray(WQ), np.asarray(WK),
        np.asarray(WV), np.asarray(WO),
    )
    res = run_bass_kernel_spmd(nc, in_maps, list(range(8)), trace=_trace)
    out = np.zeros((B, S, D), np.float32)
    for b in range(B):
        ix = idxs[b]
        yv = res.results[2 * b]["y"][: len(ix)] + res.results[2 * b + 1]["y"][: len(ix)]
        out[b][ix] = np.abs(yv)
    if _want_results:
        return out, res
    return out
